# revision 11
# baseline (speedup 1.0000x reference)
"""Trainium2 Bass kernel for NaiveRNN.

Reference computation:
    xi = x @ W_i2h + b_i2h                    # [B, L, D_h]
    h_{t+1} = tanh(xi_t + h_t @ W_h2h + b_h2h)  # L sequential steps
    out = h_L @ W_out + b_out                 # [B, D_out]

Sharding: data-parallel over batch B=128 across 8 cores (16 rows each).
Weights replicated. No cross-core communication.

Per-core kernel structure:
  Phase 1 (fp32r matmuls): xi' = x_loc @ W_i2h + (b_i2h + b_h2h), written
      to DRAM scratch [L, B_loc, D_h] so each step's slice is contiguous.
      x tiles are PE-transposed (f32 DMA transpose unsupported).
  Phase 2: 512 recurrence steps, bf16 W/h (full 1 cyc/col PE rate; fp32r
      measured ~1.5 cyc/col). State kept transposed: hT [128, 8, 16] bf16,
      which feeds matmul lhsT directly. Each step, per 512-column half:
        z_psum = I16 @ xi_t  (fp32r identity matmul injects xi on the PE,
                              keeping DVE off the critical path)
        z_psum += hT.T @ W_h2h   (8 bf16 matmuls, W as moving operand)
        h_new = tanh(z_psum)     (ACT, PSUM -> SBUF bf16)
      then one bf16 DMA-transpose (16x128 XBAR) per half turns h_new
      [16, 512] into hT [128, 4, 16] for the next step - no PE
      transposes, no DVE work on the critical path.
  Phase 3: out = h_L @ W_out + b_out (bf16 + fp32r bias matmul).
"""

import numpy as np

B, L, D_IN, D_H, D_OUT = 128, 512, 512, 1024, 512
NCORES = 8
BL = B // NCORES            # 16 local batch rows
KI = D_IN // 128            # 4 k-chunks for input proj
KH = D_H // 128             # 8 k-chunks for recurrence
ROW_TILES = (BL * L) // 128  # 64 row tiles in phase 1
LW = L // 128               # l-windows per batch row group (4)


def build_nc(l_steps=L):
    import concourse.bass as bass
    import concourse.mybir as mybir
    from concourse import bacc
    from concourse.tile import TileContext
    from concourse.masks import make_identity

    dt = mybir.dt
    f32, f32r, bf16 = dt.float32, dt.float32r, dt.bfloat16
    AF = mybir.ActivationFunctionType
    ALU = mybir.AluOpType

    nc = bacc.Bacc(
        "TRN2", target_bir_lowering=False, debug=False, num_devices=NCORES
    )
    x = nc.dram_tensor("x", [BL * L, D_IN], f32, kind="ExternalInput")
    W_i2h = nc.dram_tensor("W_i2h", [D_IN, D_H], f32, kind="ExternalInput")
    b_i2h = nc.dram_tensor("b_i2h", [D_H], f32, kind="ExternalInput")
    W_h2h = nc.dram_tensor("W_h2h", [D_H, D_H], f32, kind="ExternalInput")
    b_h2h = nc.dram_tensor("b_h2h", [D_H], f32, kind="ExternalInput")
    W_out = nc.dram_tensor("W_out", [D_H, D_OUT], f32, kind="ExternalInput")
    b_out = nc.dram_tensor("b_out", [D_OUT], f32, kind="ExternalInput")
    out = nc.dram_tensor("out", [BL, D_OUT], f32, kind="ExternalOutput")
    # NB: keep xi_dram plain f32 — float32r-typed DMAs on the sync
    # (HWDGE) ring corrupt subsequent DMA-transposes (HW-reproduced).
    # The per-step load casts f32 -> f32r on the gpsimd (SWDGE) ring.
    xi_dram = nc.dram_tensor(
        "xi_scratch", [L, BL, D_H], f32, kind="Internal"
    )

    with TileContext(nc) as tc:
        with tc.tile_pool(name="const", bufs=1) as cpool:
            # Persistent weights/constants in SBUF. gpsimd DMA casts
            # f32 -> f32r / bf16 during the load.
            whh = cpool.tile([128, KH, D_H], bf16, tag="whh")
            wi2h = cpool.tile([128, KI, D_H], f32r, tag="wi2h")
            wout = cpool.tile([128, KH, D_OUT], bf16, tag="wout")
            nc.gpsimd.dma_start(
                whh[:], W_h2h.ap().rearrange("(ko p) n -> p ko n", p=128)
            )
            nc.gpsimd.dma_start(
                wi2h[:], W_i2h.ap().rearrange("(ko p) n -> p ko n", p=128)
            )
            nc.gpsimd.dma_start(
                wout[:], W_out.ap().rearrange("(ko p) n -> p ko n", p=128)
            )
            ident = cpool.tile([128, 128], f32, tag="ident")
            make_identity(nc, ident[:])
            i16r = cpool.tile([BL, BL], f32r, tag="i16r")
            nc.vector.tensor_copy(i16r[:], ident[:BL, :BL])
            ones_f = cpool.tile([1, 128], f32, tag="ones_f")
            nc.gpsimd.memset(ones_f[:], 1.0)
            ones_row = cpool.tile([1, 128], f32r, tag="ones")
            nc.vector.tensor_copy(ones_row[:], ones_f[:])
            bi = cpool.tile([1, D_H], f32, tag="bi")
            nc.sync.dma_start(bi[:], b_i2h.ap().unsqueeze(0))
            bh = cpool.tile([1, D_H], f32, tag="bh")
            nc.sync.dma_start(bh[:], b_h2h.ap().unsqueeze(0))
            bcomb = cpool.tile([1, D_H], f32r, tag="bcomb")
            nc.vector.tensor_add(bcomb[:], bi[:], bh[:])
            bo_f = cpool.tile([1, D_OUT], f32, tag="bo_f")
            nc.sync.dma_start(bo_f[:], b_out.ap().unsqueeze(0))
            bo = cpool.tile([1, D_OUT], f32r, tag="bo")
            nc.vector.tensor_copy(bo[:], bo_f[:])

            # ---------------- Phase 1: xi' = x @ W_i2h + bcomb ----------------
            with (
                tc.tile_pool(name="p1", bufs=3) as p1pool,
                tc.tile_pool(name="p1ps_t", bufs=3, space="PSUM") as p1ps_t,
                tc.tile_pool(name="p1ps_z", bufs=2, space="PSUM") as p1ps_z,
            ):
                for r in range(ROW_TILES):
                    b_idx = r // LW
                    lw = r % LW
                    xrow = p1pool.tile([128, D_IN], f32, tag="xrow")
                    nc.sync.dma_start(
                        xrow[:], x[128 * r : 128 * r + 128, :]
                    )
                    xT = p1pool.tile([128, KI, 128], f32r, tag="xT")
                    for j in range(KI):
                        xTps = p1ps_t.tile([128, 128], f32, tag="xTps")
                        nc.tensor.transpose(
                            xTps[:], xrow[:, 128 * j : 128 * j + 128], ident[:]
                        )
                        nc.vector.tensor_copy(xT[:, j, :], xTps[:])
                    xi_sb = p1pool.tile([128, D_H], f32, tag="xi_sb")
                    for h in range(2):
                        ns = slice(512 * h, 512 * h + 512)
                        zp = p1ps_z.tile([128, 512], f32, tag="zp1")
                        for k in range(KI):
                            nc.tensor.matmul(
                                zp[:],
                                xT[:, k, :],
                                wi2h[:, k, ns],
                                start=(k == 0),
                                stop=False,
                            )
                        nc.tensor.matmul(
                            zp[:],
                            ones_row[:, :128],
                            bcomb[:, ns],
                            start=False,
                            stop=True,
                        )
                        nc.vector.tensor_copy(xi_sb[:, ns], zp[:])
                    nc.sync.dma_start(
                        xi_dram[128 * lw : 128 * lw + 128, b_idx, :], xi_sb[:]
                    )

            # ---------------- Phase 2: recurrence ----------------
            with (
                tc.tile_pool(name="p2", bufs=1) as p2pool,
                tc.tile_pool(name="p2xi", bufs=6) as xipool,
                tc.tile_pool(name="p2h", bufs=2) as hpool,
                tc.tile_pool(name="p2ps_z", bufs=2, space="PSUM") as p2ps_z,
            ):
                # state: transposed h, as 2 half-tiles (lo: dh<512, hi:
                # dh>=512) per ping-pong buffer. Ping-pong A/B so the
                # DMA-transpose write never aliases chunks the current
                # step's matmuls still read; the lo/hi split gives Tile
                # per-half dependencies so next-step matmuls k<4 start
                # while the hi-half transpose is still in flight.
                hT_a = [
                    p2pool.tile(
                        [128, KH // 2, BL], bf16, tag=f"hT_a{i}",
                        name=f"hT_a{i}",
                    )
                    for i in range(2)
                ]
                hT_b = [
                    p2pool.tile(
                        [128, KH // 2, BL], bf16, tag=f"hT_b{i}",
                        name=f"hT_b{i}",
                    )
                    for i in range(2)
                ]
                zeros_f = p2pool.tile([128, KH * BL // 2], f32, tag="zeros_f")
                nc.gpsimd.memset(zeros_f[:], 0.0)
                for i in range(2):
                    nc.vector.tensor_copy(
                        hT_a[i][:].rearrange("p a b -> p (a b)"), zeros_f[:]
                    )

                for t in range(l_steps):
                    hT_cur, hT_nxt = (
                        (hT_a, hT_b) if t % 2 == 0 else (hT_b, hT_a)
                    )
                    xi_t = xipool.tile([BL, D_H], f32r, tag="xi_t")
                    nc.gpsimd.dma_start(xi_t[:], xi_dram[t, :, :])
                    zp = p2ps_z.tile([BL, D_H], f32, tag="zp2")
                    h_new = hpool.tile([BL, D_H], bf16, tag="h_new")
                    for h in range(2):
                        ns = slice(512 * h, 512 * h + 512)
                        # inject xi_t via identity matmul (fp32r, exact)
                        nc.tensor.matmul(
                            zp[:, ns],
                            i16r[:],
                            xi_t[:, ns],
                            start=True,
                            stop=False,
                        )
                        for k in range(KH):
                            nc.tensor.matmul(
                                zp[:, ns],
                                hT_cur[k // 4][:, k % 4, :],
                                whh[:, k, ns],
                                start=False,
                                stop=(k == KH - 1),
                            )
                        nc.scalar.activation(h_new[:, ns], zp[:, ns], AF.Tanh)
                        # transpose this half into the other state buffer.
                        # Issued from the scalar (ACT) HWDGE ring: it queues
                        # right behind the tanh on the producer's own
                        # sequencer, minimizing cross-engine latency.
                        nc.scalar.dma_start(
                            hT_nxt[h][:],
                            h_new[:, ns],
                            transpose=True,
                        )

                # ---------------- Phase 3: head ----------------
                hT_fin = hT_a if l_steps % 2 == 0 else hT_b
                zp3 = p2ps_z.tile([BL, D_OUT], f32, tag="zp3")
                nc.tensor.matmul(
                    zp3[:],
                    ones_row[:, :BL],
                    bo[:],
                    start=True,
                    stop=False,
                )
                for k in range(KH):
                    nc.tensor.matmul(
                        zp3[:],
                        hT_fin[k // 4][:, k % 4, :],
                        wout[:, k, :],
                        start=False,
                        stop=(k == KH - 1),
                    )
                out_sb = p2pool.tile([BL, D_OUT], f32, tag="out_sb")
                nc.vector.tensor_copy(out_sb[:], zp3[:])
                nc.sync.dma_start(out.ap(), out_sb[:])

    nc.compile()
    return nc


_CACHE = {}


def _get_nc(l_steps=L):
    if l_steps not in _CACHE:
        _CACHE[l_steps] = build_nc(l_steps)
    return _CACHE[l_steps]


def run(inputs, l_steps=L, trace=False, tmpdir=None):
    from concourse.bass_utils import run_bass_kernel_spmd

    nc = _get_nc(l_steps)
    x = np.asarray(inputs["x"], np.float32).reshape(B, L, D_IN)
    shared = {
        k: np.ascontiguousarray(np.asarray(inputs[k], np.float32))
        for k in ("W_i2h", "b_i2h", "W_h2h", "b_h2h", "W_out", "b_out")
    }
    in_maps = []
    for c in range(NCORES):
        m = dict(shared)
        m["x"] = np.ascontiguousarray(
            x[c * BL : (c + 1) * BL].reshape(BL * L, D_IN)
        )
        in_maps.append(m)
    res = run_bass_kernel_spmd(
        nc,
        in_maps,
        core_ids=list(range(NCORES)),
        trace=trace,
        tmpdir=tmpdir,
    )
    out = np.concatenate([r["out"] for r in res.results], axis=0)
    return out, res


def kernel(**inputs) -> np.ndarray:
    out, _ = run(inputs)
    return out
